# revision 34
# baseline (speedup 1.0000x reference)
"""Trainium2 Bass kernel for the DEN-layer Mahalanobis problem.

Computes mah[b, e] = (x_b - c_e)^T Sigma_e^{-1} (x_b - c_e) for
B=8192, E=32, D=256, returning [B, E] float32.

Strategy
--------
Host precompute (cheap, E*D^2 scale):
  A_e  = Sigma_e^{-1}                    (symmetric PSD)
  L_e  = chol(A_e)      so  A_e = L_e L_e^T
  mah[b,e] = || L_e^T x_b - L_e^T c_e ||^2
           = sum_k Y[b,e,k]^2  - 2 x_b . u_e + kconst_e        (S1 trick)
  with Y = x @ L_e,  u_e = A_e c_e,  kconst_e = c_e^T A_e c_e.

Device (data parallel over B, 8 cores, B_loc=1024):
  - batched matmuls Y = x @ L_e on the PE (e's in pairs, one PSUM bank per
    pair); lower-triangular L lets us skip the zero d0->k1 block
  - square+reduce of Y straight out of PSUM, split across engines:
      * Vector: bn_stats (count/mean/M2 per e in one pass);
        sum(Y^2) = M2_even + 128*mean_even^2 + M2_odd + 128*mean_odd^2
      * Scalar: activation(Square, accum_out=...) for a minority of e's
  - tiny x@U matmul + fixup, DMA out.
Vector-handled e's sit in columns [0, 2*N_VEC_PAIR) so the bn_stats fixup
runs on one contiguous slice. Inputs are pre-transposed/packed/cast on the
host so every device DMA is contiguous.
"""

import numpy as np
import ml_dtypes

import concourse.bass as bass
import concourse.mybir as mybir
import concourse.tile as tile
from concourse.bass_utils import run_bass_kernel_spmd

E, B, D = 32, 8192, 256
N_CORES = 8
B_LOC = B // N_CORES          # 1024 rows per core
NBB = B_LOC // 128            # 8 row blocks per core
NPAIR = E // 2                # e's processed in pairs (one PSUM bank each)
P = 128

F32 = mybir.dt.float32

# Matmul operand path. float32r ("reduced" fp32, FP22 in the PE) is
# self-loading: no separate LDWEIGHTS instruction, so each matmul avoids the
# ~107ns serialized weight-load that bf16 pays, and runs 1 cycle/row at
# moving free-dim >= 256. It also carries 13 mantissa bits vs bf16's 7.
# Tiles/DRAM stay float32; APs are bitcast to float32r at the matmul.
# fp32r was tried (walrus requires fp32r-tagged producers end-to-end, works,
# rel-err 1.1e-4) but its 4-byte LDWEIGHTS costs 199ns vs bf16's 98ns per
# matmul and the weight load is serialized with the matmul in this walrus
# build (ldw-opt crashes), so bf16 is ~17us faster on the PE. bf16 rel-err
# is 3.2e-3, well within tolerance.
USE_FP32R = False
if USE_FP32R:
    MM_DT = mybir.dt.float32r
    MM_NP = np.dtype(np.float32)
else:
    MM_DT = mybir.dt.bfloat16
    MM_NP = np.dtype(ml_dtypes.bfloat16)


def _mm_ap(ap):
    return ap

# Pairs handled by the Vector engine (bn_stats) cover e in [0, 2*N_VEC_PAIR);
# vector pair j computes e=j and e=N_VEC_PAIR+j, with the two e's interleaved
# along k in the L packing so ONE bn_stats per pair yields both sums via its
# even/odd stats split. The Scalar engine (activation Square + accum) takes
# the remaining e's. Balance from measured per-e costs: bn_stats ~330ns/e
# interleaved vs activate+read-acc ~757ns/e.
N_VEC_PAIR = 11
N_VEC_E = 2 * N_VEC_PAIR
N_ACT_PAIR = NPAIR - N_VEC_PAIR


def _split_multi_waits(nc, limit=1):
    """This walrus build accepts only one sync wait per instruction
    (setupSyncWait raises "Too many sync wait commands" for >=2). Tile
    freely attaches several. Spill all but the last wait onto preceding
    single-wait NoOps on the same engine; engine program order makes this
    equivalent."""
    for fn in nc.m.functions:
        for bb in fn.blocks:
            new_list = []
            changed = False
            for inst in bb.instructions:
                si = inst.sync_info
                if si is not None and len(si.on_wait) > limit:
                    waits = list(si.on_wait)
                    for j, w in enumerate(waits[:-limit]):
                        new_list.append(
                            mybir.InstNoOp(
                                name=f"{inst.name}-ws{j}",
                                engine=inst.engine,
                                sync_info=mybir.SyncInfo(on_wait=[w], on_update=[]),
                                text_hint="waitsplit",
                                bass_nofuse=True,
                            )
                        )
                    inst.sync_info = mybir.SyncInfo(
                        on_wait=waits[-limit:], on_update=list(si.on_update)
                    )
                    changed = True
                new_list.append(inst)
            if changed:
                bb.instructions[:] = new_list


def _pair_emission_order():
    """Interleave scalar-engine pairs among vector-engine pairs."""
    vec = list(range(N_VEC_PAIR))
    act = list(range(N_VEC_PAIR, NPAIR))
    order = []
    step = max(1, len(vec) // (len(act) + 1))
    ai = 0
    for i, j in enumerate(vec):
        if ai < len(act) and i and i % (step + 1) == 0:
            order.append(act[ai])
            ai += 1
        order.append(j)
    order.extend(act[ai:])
    return order


def _build_program():
    nc = bass.Bass("TRN2", target_bir_lowering=False, debug=False,
                   num_devices=N_CORES)

    xt_d = nc.dram_tensor("xt_in", [2, P, B_LOC], MM_DT, kind="ExternalInput")
    l1_d = nc.dram_tensor("l1_in", [P, NPAIR, 512], MM_DT, kind="ExternalInput")
    l0_d = nc.dram_tensor("l0_in", [P, NPAIR, 256], MM_DT, kind="ExternalInput")
    u_d = nc.dram_tensor("u_in", [2, P, E], MM_DT, kind="ExternalInput")
    kt_d = nc.dram_tensor("kt_in", [P, E], F32, kind="ExternalInput")
    out_d = nc.dram_tensor("mah_out", [B_LOC, E], F32, kind="ExternalOutput")

    mul = mybir.AluOpType.mult
    add = mybir.AluOpType.add
    order = _pair_emission_order()

    with tile.TileContext(nc) as tc:
        with (
            tc.tile_pool(name="const", bufs=1) as const,
            tc.tile_pool(name="lw1", bufs=NPAIR) as lw1,
            tc.tile_pool(name="lw0", bufs=NPAIR) as lw0,
            tc.tile_pool(name="ypsum", bufs=5, space="PSUM") as ypsum,
            tc.tile_pool(name="xupsum", bufs=2, space="PSUM") as xupsum,
            tc.tile_pool(name="warmpsum", bufs=1, space="PSUM") as warmpsum,
            tc.tile_pool(name="scr", bufs=4) as scr,
            tc.tile_pool(name="s1p", bufs=3) as s1p,
            tc.tile_pool(name="resp", bufs=3) as resp,
        ):
            # Input loads: xt on both DMA paths in parallel, L in 4-pair
            # groups alternating between the HWDGE (sync) and SWDGE (gpsimd)
            # paths — few issue slots (~613ns each on the queue), early
            # availability of the first groups. u/kt are only needed at the
            # first block's fixup, so they load last.
            xt0 = const.tile([P, B_LOC], MM_DT, tag="xt0")
            xt1 = const.tile([P, B_LOC], MM_DT, tag="xt1")
            nc.sync.dma_start(xt0[:], xt_d[0])
            nc.gpsimd.dma_start(xt1[:], xt_d[1])

            NGRP, GSZ = 4, NPAIR // 4
            lg1, lg0 = [], []
            for g in range(NGRP):
                eng = nc.sync if g % 2 == 0 else nc.gpsimd
                t1 = lw1.tile([P, GSZ, 512], MM_DT)
                eng.dma_start(t1[:], l1_d[:, g * GSZ:(g + 1) * GSZ, :])
                lg1.append(t1)
                t0 = lw0.tile([P, GSZ, 256], MM_DT)
                eng.dma_start(t0[:], l0_d[:, g * GSZ:(g + 1) * GSZ, :])
                lg0.append(t0)
            l1_tiles = [lg1[j // GSZ][:, j % GSZ, :] for j in range(NPAIR)]
            l0_tiles = [lg0[j // GSZ][:, j % GSZ, :] for j in range(NPAIR)]

            u0 = const.tile([P, E], MM_DT, tag="u0")
            u1 = const.tile([P, E], MM_DT, tag="u1")
            nc.gpsimd.dma_start(u0[:], u_d[0])
            nc.gpsimd.dma_start(u1[:], u_d[1])
            kt = const.tile([P, E], F32, tag="kt")
            nc.gpsimd.dma_start(kt[:], kt_d[:])

            # PE warmup: throwaway matmuls on the already-loaded xt0 tile,
            # on a dedicated PSUM bank, while the L DMAs stream in — the HAM
            # clock-gate needs ~3.4us of PE activity to reach 8/8 (cold PE
            # runs at 1.2 GHz), and real matmuls can't flow until L lands.
            for _ in range(16):
                yw = warmpsum.tile([P, 512], F32, tag="yw")
                nc.tensor.matmul(yw[:, :], lhsT=_mm_ap(xt0[:, 0:P]),
                                 rhs=_mm_ap(xt0[:, 0:512]),
                                 start=True, stop=True)

            for bb in range(NBB):
                bbs = bass.ts(bb, P)
                s1 = s1p.tile([P, E], F32, tag="s1")
                stats = s1p.tile([P, N_VEC_PAIR, 6], F32, tag="stats")
                for j in order:
                    if j < N_VEC_PAIR:
                        # e=j on even k-slots, e=N_VEC_PAIR+j on odd slots.
                        y = ypsum.tile([P, 512], F32, tag="y")
                        nc.tensor.matmul(y[:, :], lhsT=_mm_ap(xt1[:, bbs]),
                                         rhs=_mm_ap(l1_tiles[j]), start=True,
                                         stop=False)
                        # d0 rows only reach k<128 (L lower-triangular):
                        # interleaved slots 2k+h, k<128 = positions [0,256)
                        nc.tensor.matmul(y[:, 0:256], lhsT=_mm_ap(xt0[:, bbs]),
                                         rhs=_mm_ap(l0_tiles[j]), start=False,
                                         stop=True)
                        nc.vector.bn_stats(stats[:, j, :], y[:, :])
                    else:
                        y = ypsum.tile([P, 2, 256], F32, tag="y")
                        nc.tensor.matmul(y[:, :, :], lhsT=_mm_ap(xt1[:, bbs]),
                                         rhs=_mm_ap(l1_tiles[j]), start=True,
                                         stop=False)
                        nc.tensor.matmul(y[:, :, 0:128], lhsT=_mm_ap(xt0[:, bbs]),
                                         rhs=_mm_ap(l0_tiles[j]), start=False,
                                         stop=True)
                        e0 = N_VEC_E + 2 * (j - N_VEC_PAIR)
                        for half, e in ((0, e0), (1, e0 + 1)):
                            sa = scr.tile([P, 256], F32, tag="sa")
                            nc.scalar.activation(
                                sa[:], y[:, half, :],
                                mybir.ActivationFunctionType.Square,
                                accum_out=s1[:, e:e + 1],
                            )
                # Vector e's from bn_stats even/odd split (n=256 each):
                #   sum(Y^2) = M2 + 256*mean^2
                m_ev, m_od = stats[:, :, 1], stats[:, :, 4]
                v_ev, v_od = stats[:, :, 2], stats[:, :, 5]
                # fixup entirely on the (otherwise idle) GpSimd engine:
                # s1 = 256*mean^2 + M2   (scalar_tensor_tensor is not
                # supported on Pool by walrus, so mul+scale+add as three TTs)
                t1_ = scr.tile([P, N_VEC_PAIR], F32, tag="fx1")
                t2_ = scr.tile([P, N_VEC_PAIR], F32, tag="fx2")
                nc.gpsimd.tensor_tensor(t1_[:], m_ev, m_ev, mul)
                nc.gpsimd.tensor_tensor(t2_[:], m_od, m_od, mul)
                nc.gpsimd.tensor_scalar_mul(t1_[:], t1_[:], 256.0)
                nc.gpsimd.tensor_scalar_mul(t2_[:], t2_[:], 256.0)
                nc.gpsimd.tensor_add(s1[:, 0:N_VEC_PAIR], t1_[:], v_ev)
                nc.gpsimd.tensor_add(s1[:, N_VEC_PAIR:N_VEC_E], t2_[:], v_od)

                xu = xupsum.tile([P, E], F32, tag="xu")
                nc.tensor.matmul(xu[:], lhsT=_mm_ap(xt0[:, bbs]), rhs=_mm_ap(u0[:]),
                                 start=True, stop=False)
                nc.tensor.matmul(xu[:], lhsT=_mm_ap(xt1[:, bbs]), rhs=_mm_ap(u1[:]),
                                 start=False, stop=True)
                res = resp.tile([P, E], F32, tag="res")
                # res = s1 - 2*xu
                nc.vector.scalar_tensor_tensor(
                    out=res[:], in0=xu[:], scalar=-2.0, in1=s1[:],
                    op0=mul, op1=add,
                )
                # res += kconst  (gpsimd: keep Vector/Scalar free)
                nc.gpsimd.tensor_add(res[:], res[:], kt[:])
                nc.sync.dma_start(out_d[bbs, :], res[:])

    _split_multi_waits(nc)
    return nc


_PROGRAM = None


def _host_prep(x, Centroids, Sigmas):
    """Returns per-core input maps (columns in device e-order)."""
    c = np.asarray(Centroids, dtype=np.float64).reshape(E, D)
    sig = np.asarray(Sigmas, dtype=np.float64)
    inv = np.linalg.inv(sig)
    inv = 0.5 * (inv + inv.transpose(0, 2, 1))
    L = np.linalg.cholesky(inv)                     # [E, D, D] lower
    u = np.einsum("edk,ek->ed", inv, c)             # [E, D]
    kconst = np.einsum("ed,ed->e", c, u)            # [E]

    # Pack L into the device layouts. Vector pair j interleaves e=j (even
    # k-slots) with e=N_VEC_PAIR+j (odd slots); Scalar pairs sit side by side.
    l1 = np.zeros((P, NPAIR, 512), dtype=np.float64)
    l0 = np.zeros((P, NPAIR, 256), dtype=np.float64)
    for j in range(N_VEC_PAIR):
        ee, eo = j, N_VEC_PAIR + j
        l1[:, j, 0::2] = L[ee, P:, :]
        l1[:, j, 1::2] = L[eo, P:, :]
        l0[:, j, 0::2] = L[ee, :P, :P]
        l0[:, j, 1::2] = L[eo, :P, :P]
    for jj in range(N_ACT_PAIR):
        j = N_VEC_PAIR + jj
        e0 = N_VEC_E + 2 * jj
        l1[:, j, 0:256] = L[e0, P:, :]
        l1[:, j, 256:512] = L[e0 + 1, P:, :]
        l0[:, j, 0:128] = L[e0, :P, :P]
        l0[:, j, 128:256] = L[e0 + 1, :P, :P]
    l1 = np.ascontiguousarray(l1).astype(MM_NP)
    l0 = np.ascontiguousarray(l0).astype(MM_NP)

    u_pack = np.ascontiguousarray(u.T.reshape(2, P, E)).astype(MM_NP)
    kt = np.broadcast_to(kconst.astype(np.float32), (P, E)).copy()

    x32 = np.asarray(x, dtype=np.float32)
    in_maps = []
    for i in range(N_CORES):
        xs = x32[i * B_LOC:(i + 1) * B_LOC]                 # [B_LOC, D]
        xt = np.ascontiguousarray(xs.T).reshape(2, P, B_LOC).astype(MM_NP)
        in_maps.append({
            "xt_in": xt,
            "l1_in": l1,
            "l0_in": l0,
            "u_in": u_pack,
            "kt_in": kt,
        })
    return in_maps


def kernel(x, Centroids, Sigmas):
    global _PROGRAM
    if _PROGRAM is None:
        _PROGRAM = _build_program()
    in_maps = _host_prep(x, Centroids, Sigmas)
    res = run_bass_kernel_spmd(_PROGRAM, in_maps, list(range(N_CORES)))
    out = np.concatenate(
        [res.results[i]["mah_out"] for i in range(N_CORES)], axis=0
    )
    return np.ascontiguousarray(out.astype(np.float32))
